# revision 16
# baseline (speedup 1.0000x reference)
"""Causal self-attention head (softmax over the QUERY axis) on 8 trn2 cores.

Reference math (softmax axis=-2, i.e. per key-column):
    q = x @ Wq; k = x @ Wk; v = x @ Wv            # [B,T,64]
    s[b,q,k] = (q . k) * 64**-0.5, masked to q >= k
    w[:, k]  = softmax over q of s[:, k]           # column softmax
    out[b,q,:] = sum_k w[q,k] v[k,:]

The softmax normalizes over q, so the normalizer folds into per-key scaling:
    out[q] = sum_{k<=q} exp(s[q,k]) * (r[k] * v[k]),  r[k] = 1/sum_{q>=k} exp(s[q,k])

Sharding: 8 cores = 4 batches x 2 "parities". Core (b, p) owns key blocks
2i+p (128 keys each); parity-1 cores get x^T pre-shifted by 128 cols
(zero-pad tail killed by a tailmask matmul); host shifts output back.

v2 kernel structure (per core, pairs j = 7..0, pair = key blocks 2j/2j+1):
- proj: ONE [Wq||Wk] matmul per contraction subtile (M=128): psum rows 0-63
  = q, rows 64-127 = k, for all 512 chunk cols. One DVE cast evacuates both;
  the core's own 2x128 key cols of k are relocated to partitions 0-63 by a
  tiny SBUF->SBUF DMA (scores need lhs/rhs on the same partitions).
- v is projected directly into natural [key, ch] layout via lhsT=x-chunk,
  rhs=Wv (N=64 matmuls) -- no DMA transposes at all.
- scores: K=64 M=128 matmuls into [128,1024] psum groups; causal diag via a
  triangular-count matmul; exp on ACT with accum_out colsums (fp32).
- output: streamed per pair with COL-TILED matmul pairs (tile_position
  (0,0)/(0,64)): even-block partial in psum rows 0-63, odd in 64-127,
  concurrently (2x PE throughput). DVE accumulates into an SBUF [128,T]
  accumulator; the even/odd halves are summed on the HOST (outT is [128,T]).
- odd blocks skip their 256 dead columns (w2 zero prefix via gpsimd memset
  instead of exp of -inf).
"""

import os
import sys
from math import ceil

import numpy as np

for _p in ("/opt/trn_rl_repo",):
    if _p not in sys.path:
        sys.path.insert(0, _p)

import concourse.bass as bass
import concourse.mybir as mybir
from concourse import bacc
from concourse.bass_utils import run_bass_kernel_spmd
from concourse.tile import TileContext

B, T, CE, CH = 4, 4096, 1024, 64
P = 128
NB = 16          # key blocks per core (128 keys each)
NP = 8           # pairs (512-col chunks)
SCALE = CH ** -0.5
NEG = -1e30
M0 = NEG / P     # per-unit magnitude for the triangular-count mask
ETILE = 1024     # scores psum group width (2 banks)

F32 = mybir.dt.float32
BF16 = mybir.dt.bfloat16

N_CORES = 8

LAST_RESULTS = None


def _build_program():
    nc = bacc.Bacc("TRN2", target_bir_lowering=False, debug=False)

    # x pre-permuted on the host: xpre[j, p, o*512+f] = x^T[o*128+p, 512j+f]
    # so each 512-col chunk is a single contiguous 1 MB HBM burst.
    xpre = nc.declare_dram_parameter("xpre", [NP, P, CE // P * 512], BF16, isOutput=False)
    wqk = nc.declare_dram_parameter("wqk", [CE, P], BF16, isOutput=False)
    wv = nc.declare_dram_parameter("wv", [CE, CH], BF16, isOutput=False)
    tailmask = nc.declare_dram_parameter("tailmask", [P, P], BF16, isOutput=False)
    outT = nc.declare_dram_parameter("outT", [P, T], F32, isOutput=True)

    with TileContext(nc) as tc:
        with (
            tc.tile_pool(name="consts", bufs=1) as consts,
            tc.tile_pool(name="qkv", bufs=1) as qkv,
            tc.tile_pool(name="w2p", bufs=1) as w2p,
            tc.tile_pool(name="xp", bufs=3) as xp,
            tc.tile_pool(name="pp", bufs=1, space="PSUM") as pp,
            tc.tile_pool(name="sp", bufs=2, space="PSUM") as sp,
            tc.tile_pool(name="op", bufs=2, space="PSUM") as op,
        ):
            # ---- DMA'd constants ----
            wqk_sb = consts.tile([P, CE // P, P], BF16, tag="wqk")
            wv_sb = consts.tile([P, CE // P, CH], BF16, tag="wv")
            nc.sync.dma_start(wqk_sb[:], wqk.rearrange("(o p) f -> p o f", p=P))
            nc.sync.dma_start(wv_sb[:], wv.rearrange("(o p) f -> p o f", p=P))
            tmask = consts.tile([P, P], BF16, tag="tmask")
            nc.sync.dma_start(tmask[:], tailmask[:])

            # ---- gpsimd-built mask constants ----
            # atri[ch, p] = 1 if ch < p; bneg[ch, c] = M0 if c <= ch
            # => (atri^T @ bneg)[p, c] = M0 * max(0, p - c)
            ones = consts.tile([P, P], BF16, tag="ones")
            nc.gpsimd.memset(ones[:], 1.0)
            atri = consts.tile([P, P], BF16, tag="atri")
            nc.gpsimd.memset(atri[:], 1.0)
            nc.gpsimd.affine_select(
                out=atri[:],
                in_=atri[:],
                compare_op=mybir.AluOpType.is_ge,
                fill=0.0,
                base=-1,
                pattern=[[1, P]],
                channel_multiplier=-1,
            )
            bneg = consts.tile([P, P], BF16, tag="bneg")
            nc.gpsimd.memset(bneg[:], M0)
            nc.gpsimd.affine_select(
                out=bneg[:],
                in_=bneg[:],
                compare_op=mybir.AluOpType.is_ge,
                fill=0.0,
                base=0,
                pattern=[[-1, P]],
                channel_multiplier=1,
            )

            # ---- persistent activations ----
            kq_sb = qkv.tile([P, T], BF16, tag="kq_sb")    # rows 0-63 q, 64-127 k-stage
            kTl = qkv.tile([CH, NB * P], BF16, tag="kTl")  # k blocks at parts 0-63
            vnat = qkv.tile([P, NB, CH], F32, tag="vnat")
            vsc = qkv.tile([P, NB, CH], BF16, tag="vsc")
            stats = qkv.tile([P, NB, 4], F32, tag="stats")
            ssum = qkv.tile([P, NB], F32, tag="ssum")
            rr = qkv.tile([P, NB], F32, tag="rr")
            outacc = qkv.tile([P, T], F32, tag="outacc")

            w2 = [
                w2p.tile([P, T - 512 * (i // 2)], BF16, tag=f"w2_{i}", name=f"w2_{i}")
                for i in range(NB)
            ]
            # odd blocks: first 256 cols are a zero prefix (dead causal zone)
            for i in range(1, NB, 2):
                nc.gpsimd.memset(w2[i][:, 0:256], 0.0)

            # A few PE warm-up matmuls while the first input DMAs land (the
            # HAM clock-gate needs sustained activity; a long spam train
            # would head-of-line-block real work, so keep it short).
            for t in range(8):
                dscr = op.tile([P, 512], F32, tag="po", name=f"warm{t}")
                nc.tensor.matmul(
                    dscr[:, 0:P], ones[:, 0:P], ones[:, 0:P],
                    start=True, stop=True,
                )
            dscr = op.tile([P, 512], F32, tag="po", name="abs_tm")
            nc.tensor.matmul(
                dscr[0:1, 0:1], tmask[:, 0:1], tmask[:, 0:1],
                start=True, stop=True,
            )

            def emit_block(i):
                j = i // 2
                odd = i % 2
                qlo = 512 * j + 256 * odd   # first live q col for this block
                L = T - qlo                  # number of exp cols
                woff = 256 * odd             # col in w2[i] where q=qlo lands
                lhs = kTl[:, P * i : P * (i + 1)]
                ngr = ceil(L / ETILE)
                for g in range(ngr):
                    gw = min(ETILE, L - ETILE * g)
                    sc = sp.tile([P, ETILE], F32, tag="sc")
                    nsub = ceil(gw / 512)
                    for u in range(nsub):
                        wu = min(512, gw - 512 * u)
                        qs = qlo + ETILE * g + 512 * u
                        # bank u gets the diag mask iff (g==0 and u==0);
                        # the tail-kill iff last group and u is last bank
                        has_diag = (g == 0 and u == 0)
                        has_tail = (g == ngr - 1 and u == nsub - 1)
                        nc.tensor.matmul(
                            sc[:, 512 * u : 512 * u + wu],
                            lhs,
                            kq_sb[0:CH, qs : qs + wu],
                            start=True,
                            stop=not (has_diag or has_tail),
                            skip_group_check=True,
                        )
                        if has_diag:
                            nc.tensor.matmul(
                                sc[:, 0:P],
                                atri[:],
                                bneg[:],
                                start=False,
                                stop=not has_tail,
                                skip_group_check=True,
                            )
                        if has_tail:
                            nc.tensor.matmul(
                                sc[:, gw - P : gw],
                                ones[:],
                                tmask[:],
                                start=False,
                                stop=True,
                                skip_group_check=True,
                            )
                    nc.scalar.activation(
                        w2[i][:, woff + ETILE * g : woff + ETILE * g + gw],
                        sc[:, :gw],
                        mybir.ActivationFunctionType.Exp,
                        scale=SCALE,
                        accum_out=stats[:, i, g : g + 1],
                    )
                nc.vector.reduce_sum(
                    ssum[:, i : i + 1],
                    stats[:, i, 0:ngr],
                    axis=mybir.AxisListType.X,
                )
                nc.vector.reciprocal(rr[:, i : i + 1], ssum[:, i : i + 1])
                nc.vector.tensor_scalar_mul(
                    vsc[:, i, :], vnat[:, i, :], rr[:, i : i + 1]
                )

            # ---- streamed output for pair j (col-tiled even/odd) ----
            # Emitted one iteration AFTER pair j's exp chain so the PE queue
            # (strict FIFO) never stalls on the ACT->rr->vsc dependency: by
            # the time the PE reaches these matmuls, vsc[j] is long done.
            def emit_output(j):
                for t in range(NP - j):
                    o = 512 * j + 512 * t
                    po = op.tile([P, 512], F32, tag="po", name=f"po{j}_{t}")
                    nc.tensor.matmul(
                        po[0:CH, :],
                        vsc[:, 2 * j, :],
                        w2[2 * j][:, 512 * t : 512 * t + 512],
                        start=True,
                        stop=True,
                        skip_group_check=True,
                    )
                    nc.tensor.matmul(
                        po[CH:P, :],
                        vsc[:, 2 * j + 1, :],
                        w2[2 * j + 1][:, 512 * t : 512 * t + 512],
                        start=True,
                        stop=True,
                        skip_group_check=True,
                    )
                    if t == 0:
                        nc.vector.tensor_copy(outacc[:, o : o + 512], po[:])
                    else:
                        nc.vector.scalar_tensor_tensor(
                            outacc[:, o : o + 512],
                            po[:],
                            1.0,
                            outacc[:, o : o + 512],
                            mybir.AluOpType.bypass,
                            mybir.AluOpType.add,
                        )
                    if j == 0:
                        # region o is final once pair 0 lands; stream it out
                        dma_eng = nc.sync if t % 2 == 0 else nc.gpsimd
                        dma_eng.dma_start(
                            outT[:, o : o + 512], outacc[:, o : o + 512]
                        )

            def emit_proj(j):
                xtile = xp.tile([P, CE // P, 512], BF16, tag="xtile")
                # split the 1MB chunk across two queues for 2x transfer bw
                nc.sync.dma_start(
                    xtile[:, 0:4, :],
                    xpre[j, :, 0 : 4 * 512].rearrange("p (o f) -> p o f", o=4),
                )
                nc.gpsimd.dma_start(
                    xtile[:, 4:8, :],
                    xpre[j, :, 4 * 512 : 8 * 512].rearrange("p (o f) -> p o f", o=4),
                )
                # absorber: put this chunk's DMA wait on a throwaway MM
                dscr = op.tile([P, 512], F32, tag="po", name=f"absx{j}")
                nc.tensor.matmul(
                    dscr[0:1, 0:1],
                    xtile[:, 0, 0:1],
                    xtile[:, 0, 0:1],
                    start=True,
                    stop=True,
                )

                # q||k projection: psum rows 0-63 = q, 64-127 = k (512 cols)
                kqps = pp.tile([P, 512], F32, tag="kqps")
                for s in range(CE // P):
                    nc.tensor.matmul(
                        kqps[:],
                        wqk_sb[:, s, :],
                        xtile[:, s, :],
                        start=(s == 0),
                        stop=(s == CE // P - 1),
                    )
                nc.vector.tensor_copy(kq_sb[:, 512 * j : 512 * (j + 1)], kqps[:])
                # relocate own k cols (0:128, 256:384 of chunk) to parts 0-63.
                # On the SCALAR queue: by the time the ACT stream reaches this
                # descriptor, its wait (the kq cast, 2 pairs ahead of the exp
                # stream) is already satisfied — unlike on the sync/gpsimd
                # queues, where the wait would head-of-line-block the chunk
                # transfers behind it.
                nc.scalar.dma_start(
                    kTl[:, 256 * j : 256 * (j + 1)].rearrange(
                        "p (b c) -> p b c", c=P
                    ),
                    kq_sb[CH:P, 512 * j : 512 * (j + 1)].rearrange(
                        "p (b c) -> p b c", c=P
                    )[:, 0::2, :],
                )

                # v directly in natural [key, ch] layout: lhsT = x key cols
                vps = pp.tile([P, P], F32, tag="vps")
                for s in range(CE // P):
                    nc.tensor.matmul(
                        vps[:, 0:CH],
                        xtile[:, s, 0:P],
                        wv_sb[:, s, :],
                        start=(s == 0),
                        stop=(s == CE // P - 1),
                        skip_group_check=True,
                    )
                for s in range(CE // P):
                    nc.tensor.matmul(
                        vps[:, CH:P],
                        xtile[:, s, 256 : 256 + P],
                        wv_sb[:, s, :],
                        start=(s == 0),
                        stop=(s == CE // P - 1),
                        skip_group_check=True,
                    )
                nc.vector.tensor_copy(
                    vnat[:, 2 * j : 2 * j + 2, :].rearrange("p b c -> p (b c)"),
                    vps[:],
                )

            # ======== pipeline: pairs descending, proj runs 2 pairs ahead ====
            emit_proj(NP - 1)
            emit_proj(NP - 2)
            for j in reversed(range(NP)):
                emit_block(2 * j)
                emit_block(2 * j + 1)
                if j >= 2:
                    emit_proj(j - 2)
                if j < NP - 1:
                    emit_output(j + 1)
            emit_output(0)

    return nc


_PROGRAM = None


def _get_program():
    global _PROGRAM
    if _PROGRAM is None:
        nc = _build_program()
        nc.finalize()
        _PROGRAM = nc
    return _PROGRAM


def kernel(x, Wk, Wq, Wv, trace=False, trace_cores=None):
    global LAST_RESULTS
    x = np.asarray(x)
    Wk = np.asarray(Wk)
    Wq = np.asarray(Wq)
    Wv = np.asarray(Wv)

    import ml_dtypes

    bf = ml_dtypes.bfloat16
    wqk_b = np.concatenate([Wq, Wk], axis=1).astype(bf)  # [CE, 128]
    wv_b = Wv.astype(bf)

    zeros_mask = np.zeros((P, P), bf)
    neg_mask = np.full((P, P), NEG / P, bf)

    in_maps = []
    for c in range(N_CORES):
        b, parity = c // 2, c % 2
        xTb = np.ascontiguousarray(x[b].T).astype(bf)  # [CE, T]
        if parity:
            xTb = np.concatenate([xTb[:, P:], np.zeros((CE, P), bf)], axis=1)
        # xpre[j, p, o*512+f] = xT[o*128+p, 512j+f]: contiguous per chunk
        xpre = (
            xTb.reshape(CE // P, P, NP, 512)
            .transpose(2, 1, 0, 3)
            .reshape(NP, P, CE // P * 512)
        )
        in_maps.append(
            {
                "xpre": np.ascontiguousarray(xpre),
                "wqk": wqk_b,
                "wv": wv_b,
                "tailmask": neg_mask if parity else zeros_mask,
            }
        )

    nc = _get_program()
    res = run_bass_kernel_spmd(
        nc,
        in_maps,
        list(range(N_CORES)),
        trace=trace,
        **({"trace_cores": trace_cores} if trace_cores is not None else {}),
    )
    LAST_RESULTS = res

    out = np.zeros((B, T, CH), np.float32)
    for c in range(N_CORES):
        b, parity = c // 2, c % 2
        oTf = np.asarray(res.results[c]["outT"], np.float32)  # [128, T]
        oT = oTf[0:CH] + oTf[CH:P]  # fold even/odd block halves
        if parity:
            out[b, P:, :] += oT[:, : T - P].T
        else:
            out[b] += oT.T
    return out


# revision 21
# speedup vs baseline: 1.0928x; 1.0928x over previous
"""Causal self-attention head (softmax over the QUERY axis) on 8 trn2 cores.

Reference math (softmax axis=-2, i.e. per key-column):
    q = x @ Wq; k = x @ Wk; v = x @ Wv            # [B,T,64]
    s[b,q,k] = (q . k) * 64**-0.5, masked to q >= k
    w[:, k]  = softmax over q of s[:, k]           # column softmax
    out[b,q,:] = sum_k w[q,k] v[k,:]

The softmax normalizes over q, so the normalizer folds into per-key scaling:
    out[q] = sum_{k<=q} exp(s[q,k]) * (r[k] * v[k]),  r[k] = 1/sum_{q>=k} exp(s[q,k])

Sharding: 8 cores = 4 batches x 2 "parities". Core (b, p) owns key blocks
2i+p (128 keys each); parity-1 cores get x^T pre-shifted by 128 cols
(zero-pad tail killed by a tailmask matmul); host shifts output back.

v2 kernel structure (per core, pairs j = 7..0, pair = key blocks 2j/2j+1):
- proj: ONE [Wq||Wk] matmul per contraction subtile (M=128): psum rows 0-63
  = q, rows 64-127 = k, for all 512 chunk cols. One DVE cast evacuates both;
  the core's own 2x128 key cols of k are relocated to partitions 0-63 by a
  tiny SBUF->SBUF DMA (scores need lhs/rhs on the same partitions).
- v is projected directly into natural [key, ch] layout via lhsT=x-chunk,
  rhs=Wv (N=64 matmuls) -- no DMA transposes at all.
- scores: K=64 M=128 matmuls into [128,1024] psum groups; causal diag via a
  triangular-count matmul; exp on ACT with accum_out colsums (fp32).
- output: streamed per pair with COL-TILED matmul pairs (tile_position
  (0,0)/(0,64)): even-block partial in psum rows 0-63, odd in 64-127,
  concurrently (2x PE throughput). DVE accumulates into an SBUF [128,T]
  accumulator; the even/odd halves are summed on the HOST (outT is [128,T]).
- odd blocks skip their 256 dead columns (w2 zero prefix via gpsimd memset
  instead of exp of -inf).
"""

import os
import sys
from math import ceil

import numpy as np

for _p in ("/opt/trn_rl_repo",):
    if _p not in sys.path:
        sys.path.insert(0, _p)

import concourse.bass as bass
import concourse.mybir as mybir
from concourse import bacc
from concourse.bass_utils import run_bass_kernel_spmd
from concourse.tile import TileContext

B, T, CE, CH = 4, 4096, 1024, 64
P = 128
NB = 16          # key blocks per core (128 keys each)
NP = 8           # pairs (512-col chunks)
SCALE = CH ** -0.5
NEG = -1e30
M0 = NEG / P     # per-unit magnitude for the triangular-count mask
ETILE = 1024     # scores psum group width (2 banks)

F32 = mybir.dt.float32
BF16 = mybir.dt.bfloat16

N_CORES = 8

LAST_RESULTS = None


def _build_program():
    nc = bacc.Bacc("TRN2", target_bir_lowering=False, debug=False)

    # x pre-permuted on the host: xpre[j, p, o*512+f] = x^T[o*128+p, 512j+f]
    # so each 512-col chunk is a single contiguous 1 MB HBM burst.
    xpre = nc.declare_dram_parameter("xpre", [NP, P, CE // P * 512], BF16, isOutput=False)
    wq = nc.declare_dram_parameter("wq", [CE, CH], BF16, isOutput=False)
    wk = nc.declare_dram_parameter("wk", [CE, CH], BF16, isOutput=False)
    wv = nc.declare_dram_parameter("wv", [CE, CH], BF16, isOutput=False)
    tailmask = nc.declare_dram_parameter("tailmask", [P, P], BF16, isOutput=False)
    outT = nc.declare_dram_parameter("outT", [P, T], F32, isOutput=True)

    with TileContext(nc) as tc:
        with (
            tc.tile_pool(name="consts", bufs=1) as consts,
            tc.tile_pool(name="qkv", bufs=1) as qkv,
            tc.tile_pool(name="w2p", bufs=1) as w2p,
            tc.tile_pool(name="xp", bufs=3) as xp,
            tc.tile_pool(name="pp", bufs=1, space="PSUM") as pp,
            tc.tile_pool(name="sp", bufs=2, space="PSUM") as sp,
            tc.tile_pool(name="op", bufs=2, space="PSUM") as op,
        ):
            # ---- DMA'd constants ----
            wq_sb = consts.tile([P, CE // P, CH], BF16, tag="wq")
            wk_sb = consts.tile([P, CE // P, CH], BF16, tag="wk")
            wv_sb = consts.tile([P, CE // P, CH], BF16, tag="wv")
            nc.sync.dma_start(wq_sb[:], wq.rearrange("(o p) f -> p o f", p=P))
            nc.sync.dma_start(wk_sb[:], wk.rearrange("(o p) f -> p o f", p=P))
            nc.sync.dma_start(wv_sb[:], wv.rearrange("(o p) f -> p o f", p=P))
            tmask = consts.tile([P, P], BF16, tag="tmask")
            nc.sync.dma_start(tmask[:], tailmask[:])

            # ---- gpsimd-built mask constants ----
            # atri[ch, p] = 1 if ch < p; bneg[ch, c] = M0 if c <= ch
            # => (atri^T @ bneg)[p, c] = M0 * max(0, p - c)
            ones = consts.tile([P, P], BF16, tag="ones")
            nc.gpsimd.memset(ones[:], 1.0)
            atri = consts.tile([P, P], BF16, tag="atri")
            nc.gpsimd.memset(atri[:], 1.0)
            nc.gpsimd.affine_select(
                out=atri[:],
                in_=atri[:],
                compare_op=mybir.AluOpType.is_ge,
                fill=0.0,
                base=-1,
                pattern=[[1, P]],
                channel_multiplier=-1,
            )
            bneg = consts.tile([P, P], BF16, tag="bneg")
            nc.gpsimd.memset(bneg[:], M0)
            nc.gpsimd.affine_select(
                out=bneg[:],
                in_=bneg[:],
                compare_op=mybir.AluOpType.is_ge,
                fill=0.0,
                base=0,
                pattern=[[-1, P]],
                channel_multiplier=1,
            )

            # ---- persistent activations ----
            qT = qkv.tile([CH, T], BF16, tag="qT")         # q at parts 0-63
            kTl = qkv.tile([CH, NB * P], BF16, tag="kTl")  # k blocks at parts 0-63
            vnat = qkv.tile([P, NB, CH], F32, tag="vnat")
            vsc = qkv.tile([P, NB, CH], BF16, tag="vsc")
            stats = qkv.tile([P, NB, 4], F32, tag="stats")
            ssum = qkv.tile([P, NB], F32, tag="ssum")
            rr = qkv.tile([P, NB], F32, tag="rr")
            outacc = qkv.tile([P, T], F32, tag="outacc")

            w2 = [
                w2p.tile([P, T - 512 * (i // 2)], BF16, tag=f"w2_{i}", name=f"w2_{i}")
                for i in range(NB)
            ]
            # odd blocks: first 256 cols are a zero prefix (dead causal zone)
            for i in range(1, NB, 2):
                nc.gpsimd.memset(w2[i][:, 0:256], 0.0)

            # A few PE warm-up matmuls while the first input DMAs land (the
            # HAM clock-gate needs sustained activity; a long spam train
            # would head-of-line-block real work, so keep it short).
            for t in range(32):
                dscr = op.tile([P, 512], F32, tag="po", name=f"warm{t}")
                nc.tensor.matmul(
                    dscr[:, 0:P], ones[:, 0:P], ones[:, 0:P],
                    start=True, stop=True,
                )
            dscr = op.tile([P, 512], F32, tag="po", name="abs_tm")
            nc.tensor.matmul(
                dscr[0:1, 0:1], tmask[:, 0:1], tmask[:, 0:1],
                start=True, stop=True,
            )

            def emit_block(i):
                j = i // 2
                odd = i % 2
                qlo = 512 * j + 256 * odd   # first live q col for this block
                L = T - qlo                  # number of exp cols
                woff = 256 * odd             # col in w2[i] where q=qlo lands
                lhs = kTl[:, P * i : P * (i + 1)]
                ngr = ceil(L / ETILE)
                for g in range(ngr):
                    gw = min(ETILE, L - ETILE * g)
                    sc = sp.tile([P, ETILE], F32, tag="sc")
                    nsub = ceil(gw / 512)
                    for u in range(nsub):
                        wu = min(512, gw - 512 * u)
                        qs = qlo + ETILE * g + 512 * u
                        # bank u gets the diag mask iff (g==0 and u==0);
                        # the tail-kill iff last group and u is last bank
                        has_diag = (g == 0 and u == 0)
                        has_tail = (g == ngr - 1 and u == nsub - 1)
                        nc.tensor.matmul(
                            sc[:, 512 * u : 512 * u + wu],
                            lhs,
                            qT[:, qs : qs + wu],
                            start=True,
                            stop=not (has_diag or has_tail),
                            skip_group_check=True,
                        )
                        if has_diag:
                            nc.tensor.matmul(
                                sc[:, 0:P],
                                atri[:],
                                bneg[:],
                                start=False,
                                stop=not has_tail,
                                skip_group_check=True,
                            )
                        if has_tail:
                            nc.tensor.matmul(
                                sc[:, gw - P : gw],
                                ones[:],
                                tmask[:],
                                start=False,
                                stop=True,
                                skip_group_check=True,
                            )
                    nc.scalar.activation(
                        w2[i][:, woff + ETILE * g : woff + ETILE * g + gw],
                        sc[:, :gw],
                        mybir.ActivationFunctionType.Exp,
                        scale=SCALE,
                        accum_out=stats[:, i, g : g + 1],
                    )
                nc.vector.reduce_sum(
                    ssum[:, i : i + 1],
                    stats[:, i, 0:ngr],
                    axis=mybir.AxisListType.X,
                )
                nc.vector.reciprocal(rr[:, i : i + 1], ssum[:, i : i + 1])
                nc.vector.tensor_scalar_mul(
                    vsc[:, i, :], vnat[:, i, :], rr[:, i : i + 1]
                )

            # ---- streamed output for pair j (col-tiled even/odd) ----
            # Emitted one iteration AFTER pair j's exp chain so the PE queue
            # (strict FIFO) never stalls on the ACT->rr->vsc dependency: by
            # the time the PE reaches these matmuls, vsc[j] is long done.
            def emit_output(j):
                for t in range(NP - j):
                    o = 512 * j + 512 * t
                    po = op.tile([P, 512], F32, tag="po", name=f"po{j}_{t}")
                    nc.tensor.matmul(
                        po[0:CH, :],
                        vsc[:, 2 * j, :],
                        w2[2 * j][:, 512 * t : 512 * t + 512],
                        start=True,
                        stop=True,
                        skip_group_check=True,
                    )
                    nc.tensor.matmul(
                        po[CH:P, :],
                        vsc[:, 2 * j + 1, :],
                        w2[2 * j + 1][:, 512 * t : 512 * t + 512],
                        start=True,
                        stop=True,
                        skip_group_check=True,
                    )
                    if t == 0:
                        nc.vector.tensor_copy(outacc[:, o : o + 512], po[:])
                    else:
                        nc.vector.scalar_tensor_tensor(
                            outacc[:, o : o + 512],
                            po[:],
                            1.0,
                            outacc[:, o : o + 512],
                            mybir.AluOpType.bypass,
                            mybir.AluOpType.add,
                        )
                    if j == 0:
                        # region o is final once pair 0 lands; stream it out
                        dma_eng = nc.sync if t % 2 == 0 else nc.gpsimd
                        dma_eng.dma_start(
                            outT[:, o : o + 512], outacc[:, o : o + 512]
                        )

            def emit_proj(j):
                xtile = xp.tile([P, CE // P, 512], BF16, tag="xtile")
                # split the 1MB chunk across two queues for 2x transfer bw
                nc.sync.dma_start(
                    xtile[:, 0:4, :],
                    xpre[j, :, 0 : 4 * 512].rearrange("p (o f) -> p o f", o=4),
                )
                nc.gpsimd.dma_start(
                    xtile[:, 4:8, :],
                    xpre[j, :, 4 * 512 : 8 * 512].rearrange("p (o f) -> p o f", o=4),
                )
                # absorber: put this chunk's DMA wait on a throwaway MM
                dscr = op.tile([P, 512], F32, tag="po", name=f"absx{j}")
                nc.tensor.matmul(
                    dscr[0:1, 0:1],
                    xtile[:, 0, 0:1],
                    xtile[:, 0, 0:1],
                    start=True,
                    stop=True,
                )

                # q projection: [64, 512] psum at parts 0-63, straight cast
                qps = pp.tile([CH, 512], F32, tag="qps")
                for s in range(CE // P):
                    nc.tensor.matmul(
                        qps[:],
                        wq_sb[:, s, :],
                        xtile[:, s, :],
                        start=(s == 0),
                        stop=(s == CE // P - 1),
                    )
                nc.vector.tensor_copy(qT[:, 512 * j : 512 * (j + 1)], qps[:])

                # k (own 2x128 key cols) and v (natural layout) share a bank:
                # kvps[:, 0:128] = v (keys x ch for both blocks),
                # kvps[0:64, 128:384] = kT for both blocks.
                kvps = pp.tile([P, 384], F32, tag="kvps")
                for s in range(CE // P):
                    kvrhs = xtile[:, s, :].rearrange("p (b c) -> p b c", c=P)[
                        :, 0::2, :
                    ]
                    nc.tensor.matmul(
                        kvps[0:CH, 128:384].rearrange("p (b c) -> p b c", c=P),
                        wk_sb[:, s, :],
                        kvrhs,
                        start=(s == 0),
                        stop=(s == CE // P - 1),
                        skip_group_check=True,
                    )
                for s in range(CE // P):
                    nc.tensor.matmul(
                        kvps[:, 0:CH],
                        xtile[:, s, 0:P],
                        wv_sb[:, s, :],
                        start=(s == 0),
                        stop=(s == CE // P - 1),
                        skip_group_check=True,
                    )
                for s in range(CE // P):
                    nc.tensor.matmul(
                        kvps[:, CH:P],
                        xtile[:, s, 256 : 256 + P],
                        wv_sb[:, s, :],
                        start=(s == 0),
                        stop=(s == CE // P - 1),
                        skip_group_check=True,
                    )
                nc.vector.tensor_copy(
                    vnat[:, 2 * j : 2 * j + 2, :].rearrange("p b c -> p (b c)"),
                    kvps[:, 0:P],
                )
                nc.vector.tensor_copy(
                    kTl[:, 256 * j : 256 * (j + 1)], kvps[0:CH, 128:384]
                )

            # ======== pipeline: pairs descending, proj runs 2 pairs ahead ====
            emit_proj(NP - 1)
            emit_proj(NP - 2)
            for j in reversed(range(NP)):
                emit_block(2 * j)
                emit_block(2 * j + 1)
                if j >= 2:
                    emit_proj(j - 2)
                if j < NP - 1:
                    emit_output(j + 1)
            emit_output(0)

    return nc


_PROGRAM = None


def _get_program():
    global _PROGRAM
    if _PROGRAM is None:
        nc = _build_program()
        nc.finalize()
        _PROGRAM = nc
    return _PROGRAM


def kernel(x, Wk, Wq, Wv, trace=False, trace_cores=None):
    global LAST_RESULTS
    x = np.asarray(x)
    Wk = np.asarray(Wk)
    Wq = np.asarray(Wq)
    Wv = np.asarray(Wv)

    import ml_dtypes

    bf = ml_dtypes.bfloat16
    wq_b = Wq.astype(bf)
    wk_b = Wk.astype(bf)
    wv_b = Wv.astype(bf)

    zeros_mask = np.zeros((P, P), bf)
    neg_mask = np.full((P, P), NEG / P, bf)

    in_maps = []
    for c in range(N_CORES):
        b, parity = c // 2, c % 2
        xTb = np.ascontiguousarray(x[b].T).astype(bf)  # [CE, T]
        if parity:
            xTb = np.concatenate([xTb[:, P:], np.zeros((CE, P), bf)], axis=1)
        # xpre[j, p, o*512+f] = xT[o*128+p, 512j+f]: contiguous per chunk
        xpre = (
            xTb.reshape(CE // P, P, NP, 512)
            .transpose(2, 1, 0, 3)
            .reshape(NP, P, CE // P * 512)
        )
        in_maps.append(
            {
                "xpre": np.ascontiguousarray(xpre),
                "wq": wq_b,
                "wk": wk_b,
                "wv": wv_b,
                "tailmask": neg_mask if parity else zeros_mask,
            }
        )

    nc = _get_program()
    res = run_bass_kernel_spmd(
        nc,
        in_maps,
        list(range(N_CORES)),
        trace=trace,
        **({"trace_cores": trace_cores} if trace_cores is not None else {}),
    )
    LAST_RESULTS = res

    out = np.zeros((B, T, CH), np.float32)
    for c in range(N_CORES):
        b, parity = c // 2, c % 2
        oTf = np.asarray(res.results[c]["outT"], np.float32)  # [128, T]
        oT = oTf[0:CH] + oTf[CH:P]  # fold even/odd block halves
        if parity:
            out[b, P:, :] += oT[:, : T - P].T
        else:
            out[b] += oT.T
    return out
